# revision 1
# baseline (speedup 1.0000x reference)
"""Trainium2 Bass kernel for CompositionalTwoArmedAgent (DND-LSTM A2C step).

Strategy (8 NeuronCores, SPMD + AllReduce):
  - DND keys/vals tables sharded row-wise: 12544 rows/core (core 7 padded).
  - Cosine similarities are bounded in [-1, 1], so the softmax needs no
    max pass: each core computes e_i = exp(cos_i), a partial sum S_k and a
    partial weighted value sum p_k = e @ vals_k (TensorE, fp32r full rate).
  - The LSTM i2h/h2h GEMM is sharded over its contraction dim (128 h-dims
    per core; the x_t @ W_i2h.T part is zero-fed on cores 1..7).
  - Two AllReduces: [preact(5120) | S(1)] early (hidden under the vals
    stream, so the gate math is precomputed), p(1024) late (4 KB).
  - Every core then computes the identical tiny LSTM/A2C tail; host reads
    core 0's output, applies the 2-class softmax / fixed-key categorical
    sample, and packs the reference's output layout.
"""

import ml_dtypes
import numpy as np

import concourse.bacc as bacc
import concourse.bass as bass
import concourse.mybir as mybir
import concourse.tile as tile
from concourse.bass_utils import run_bass_kernel_spmd

N_CORES = 8
D, RD, H, IN_DIM, A = 100000, 10, 1024, 14, 2
PER = 12544            # padded rows per core = G * 128
G = 98                 # 128-row chunks per core
BLOCKS = [14] * 6 + [7, 4, 2, 1]   # chunks per vals DMA block (descending tail)
F32 = mybir.dt.float32
F32R = mybir.dt.float32r
BF16 = mybir.dt.bfloat16
F16 = mybir.dt.float16

# jax.random.gumbel(jax.random.key(1), (2,), float32) — fixed constants of the
# reference's categorical sample (verified against jax.random.categorical).
GUMBEL = np.array([0.5325072, -0.01641824], np.float32)

_CACHE = {}


def _input_specs():
    return [
        ("vals_s", [128, G * H], BF16),    # row-chunk-tiled vals shard
        ("keys_t", [128, G * RD], F32),
        ("q_rep", [128, G * RD], F32),
        ("mask", [128, G], F32),
        ("wht", [128, 5 * H], F16),
        ("wxt", [IN_DIM, 5 * H], F16),
        ("x_col", [IN_DIM, 1], F16),
        ("h_col", [128, 1], F16),
        ("c2t", [128, 8], F32),
        ("b5t", [128, 40], F32),
        ("biht", [128, 8], F32),
        ("wiht", [128, 8 * H], BF16),
        ("wact", [128, 24], F32),
        ("bac", [1, 3], F32),
    ]


def _build():
    nc = bacc.Bacc("TRN2", target_bir_lowering=False, debug=False,
                   num_devices=N_CORES)
    d = {name: nc.dram_tensor(name, shp, dt, kind="ExternalInput")
         for name, shp, dt in _input_specs()}
    out_hc = nc.dram_tensor("out_hc", [128, 16], F32, kind="ExternalOutput")
    out_av = nc.dram_tensor("out_av", [1, 3], F32, kind="ExternalOutput")

    AF = mybir.ActivationFunctionType
    OP = mybir.AluOpType

    with tile.TileContext(nc) as tc:
        with (
            tc.tile_pool(name="const", bufs=1) as cp,
            tc.tile_pool(name="vals", bufs=3) as vp,
            tc.tile_pool(name="ps", bufs=1, space="PSUM") as pp,
            tc.tile_pool(name="dram", bufs=1, space="DRAM") as dp,
        ):
            # ---- persistent loads -------------------------------------
            keys_sb = cp.tile([128, G * RD], F32)
            q_sb = cp.tile([128, G * RD], F32)
            mask_sb = cp.tile([128, G], F32)
            wht_sb = cp.tile([128, 5 * H], F16)
            wxt_sb = cp.tile([IN_DIM, 5 * H], F16)
            x_col_sb = cp.tile([IN_DIM, 1], F16)
            h_col_sb = cp.tile([128, 1], F16)
            c2t_sb = cp.tile([128, 8], F32)
            b5t_sb = cp.tile([128, 40], F32)
            biht_sb = cp.tile([128, 8], F32)
            wiht_sb = cp.tile([128, 8, H], BF16)
            wact_sb = cp.tile([128, 24], F32)
            bac_sb = cp.tile([1, 3], F32)
            for name, t in [("wht", wht_sb), ("wxt", wxt_sb),
                            ("x_col", x_col_sb), ("h_col", h_col_sb),
                            ("keys_t", keys_sb), ("q_rep", q_sb),
                            ("mask", mask_sb), ("c2t", c2t_sb),
                            ("b5t", b5t_sb), ("biht", biht_sb),
                            ("wact", wact_sb), ("bac", bac_sb)]:
                nc.scalar.dma_start(t[:], d[name][:])
            nc.scalar.dma_start(
                wiht_sb[:], d["wiht"][:].rearrange("p (c j) -> p c j", j=H))

            ones_sb = cp.tile([128, 128], F32)
            nc.vector.memset(ones_sb[:], 1.0)
            one16 = cp.tile([1, 1], F16)
            nc.vector.memset(one16[:], 1.0)

            # ---- ||q||^2 broadcast to all partitions ------------------
            sq_q = cp.tile([1, RD], F32)
            nc.scalar.activation(sq_q[:], q_sb[0:1, 0:RD], AF.Square)
            qnsq = cp.tile([1, 1], F32)
            nc.vector.reduce_sum(qnsq[:], sq_q[:], axis=mybir.AxisListType.X)
            psum_qn = pp.tile([128, 1], F32, tag="ps_small")
            nc.tensor.matmul(psum_qn[:], ones_sb[0:1, :], qnsq[:])
            qn2b = cp.tile([128, 1], F32)
            nc.vector.tensor_copy(qn2b[:], psum_qn[:])

            # ---- cosine sims -> masked exp weights --------------------
            prod = cp.tile([128, G * RD], F32)
            nc.vector.tensor_mul(prod[:], keys_sb[:], q_sb[:])
            dots = cp.tile([128, G], F32)
            nc.vector.tensor_reduce(
                dots[:], prod[:].rearrange("p (g r) -> p g r", r=RD),
                axis=mybir.AxisListType.X, op=OP.add)
            sqk = cp.tile([128, G * RD], F32)
            nc.scalar.activation(sqk[:], keys_sb[:], AF.Square)
            nsq = cp.tile([128, G], F32)
            nc.vector.tensor_reduce(
                nsq[:], sqk[:].rearrange("p (g r) -> p g r", r=RD),
                axis=mybir.AxisListType.X, op=OP.add)
            d2 = cp.tile([128, G], F32)
            nc.vector.tensor_scalar(d2[:], nsq[:], qn2b[:, 0:1], None, OP.mult)
            den = cp.tile([128, G], F32)
            nc.scalar.activation(den[:], d2[:], AF.Sqrt)
            denc = cp.tile([128, G], F32)
            nc.vector.tensor_scalar_max(denc[:], den[:], 1e-8)
            rec = cp.tile([128, G], F32)
            nc.vector.reciprocal(rec[:], denc[:])
            s_sb = cp.tile([128, G], F32)
            nc.vector.tensor_mul(s_sb[:], dots[:], rec[:])
            eraw = cp.tile([128, G], F32)
            nc.scalar.activation(eraw[:], s_sb[:], AF.Exp)
            e_sb = cp.tile([128, G], F32)
            rowsum = cp.tile([128, 1], F32)
            nc.vector.scalar_tensor_tensor(
                e_sb[:], eraw[:], 1.0, mask_sb[:], OP.mult, OP.mult,
                accum_out=rowsum[:])
            e_r = cp.tile([128, G], BF16)
            nc.vector.tensor_copy(e_r[:], e_sb[:])

            # ---- preact partial: [x;h_chunk] @ [WxT;WhT] --------------
            # moving-operand fp32r form: 20 N=512 matmuls into [1,512] rows,
            # then 40 PE transposes back to the compact [128, 40] col layout.
            psum_pre = pp.tile([128, 80], F16)
            for n in range(10):
                pre_ps = pp.tile([1, 512], F32, tag=f"pre{n % 2}")
                nc.tensor.matmul(pre_ps[:], h_col_sb[:],
                                 wht_sb[:, n * 512:(n + 1) * 512],
                                 start=True, stop=False)
                nc.tensor.matmul(pre_ps[:], x_col_sb[:],
                                 wxt_sb[:, n * 512:(n + 1) * 512],
                                 start=False, stop=True)
                row_scr = cp.tile([1, 512], F16, tag="rowscr", bufs=2)
                nc.vector.tensor_copy(row_scr[:], pre_ps[:])
                for t in range(4):
                    k = 2 * (4 * n + t)   # even fp16 col = 4-byte aligned
                    nc.tensor.transpose(psum_pre[:, k:k + 1],
                                        row_scr[0:1, t * 128:(t + 1) * 128],
                                        one16[:])

            # ---- big matvec: p = e @ vals (fp32r, streamed) -----------
            p0 = pp.tile([1, 512], F32)
            p1 = pp.tile([1, 512], F32)
            g = 0
            for nb in BLOCKS:
                v = vp.tile([128, nb, H], BF16, tag="v")
                src = d["vals_s"][:, g * H:(g + nb) * H]
                nc.sync.dma_start(v[:], src.rearrange("p (c h) -> p c h", h=H))
                for c in range(nb):
                    e_col = e_r[:, g:g + 1]
                    nc.tensor.matmul(p0[:], e_col, v[:, c, 0:512],
                                     start=(g == 0), stop=(g == G - 1))
                    nc.tensor.matmul(p1[:], e_col, v[:, c, 512:1024],
                                     start=(g == 0), stop=(g == G - 1))
                    g += 1

            # ---- transpose p to [128, 8] ------------------------------
            p_sb = cp.tile([1, H], F32)
            nc.vector.tensor_copy(p_sb[0:1, 0:512], p0[:])
            nc.vector.tensor_copy(p_sb[0:1, 512:1024], p1[:])
            psum_mt = pp.tile([128, 8], F32)
            for n in range(8):
                nc.tensor.transpose(psum_mt[:, n:n + 1],
                                    p_sb[0:1, n * 128:(n + 1) * 128],
                                    ones_sb[0:1, 0:1])

            # ---- single AllReduce: [preact(40) | p(8) | S(1)] ---------
            stage2 = cp.tile([128, 49], F32)
            nc.vector.tensor_copy(
                stage2[:, 0:40].rearrange("p (c one) -> p c one", one=1),
                psum_pre[:].rearrange("p (c two) -> p c two", two=2)[:, :, 0:1])
            nc.vector.tensor_copy(stage2[:, 48:49], rowsum[:])
            i_stage2 = nc.vector.tensor_copy(stage2[:, 40:48], psum_mt[:])
            cc2_in = dp.tile([128, 49], F32)
            cc2_out = dp.tile([128, 49], F32, addr_space="Shared")
            nc.sync.dma_start(cc2_in[:], stage2[:])
            i_cc2 = nc.gpsimd.collective_compute(
                "AllReduce", OP.add,
                replica_groups=[list(range(N_CORES))],
                ins=[cc2_in[:]], outs=[cc2_out[:]])
            stage2o = cp.tile([128, 49], F32)
            nc.sync.dma_start(stage2o[:], cc2_out[:])

            # ---- gate math from AR1 (hidden under the vals stream) ----
            prefull = cp.tile([128, 40], F32)
            i_pf = nc.vector.tensor_add(prefull[:], stage2o[:, 0:40], b5t_sb[:])
            # keep the AR1-gated DVE chain behind the AR2 staging copy so the
            # scheduler cannot stall the vector queue on AR1 completion
            tile.add_dep_helper(i_pf.ins, i_stage2.ins, sync=False,
                                reason="gate math after AR2 staging")
            th = cp.tile([128, 32], F32)
            nc.scalar.activation(th[:], prefull[:, 0:32], AF.Tanh, scale=0.5)
            gates = cp.tile([128, 32], F32)
            nc.vector.tensor_scalar(gates[:], th[:], 0.5, 0.5, OP.mult, OP.add)
            cnew = cp.tile([128, 8], F32)
            nc.scalar.activation(cnew[:], prefull[:, 32:40], AF.Tanh)
            S_all = cp.tile([128, 1], F32)
            i_sall = nc.gpsimd.partition_all_reduce(
                S_all[:], stage2o[:, 48:49], 128,
                bass.bass_isa.ReduceOp.add)
            tile.add_dep_helper(i_sall.ins, i_cc2.ins, sync=False,
                                reason="keep gpsimd doorbell ahead of S reduce")
            invS = cp.tile([128, 1], F32)
            nc.vector.reciprocal(invS[:], S_all[:])
            t1 = cp.tile([128, 8], F32)
            nc.vector.tensor_mul(t1[:], gates[:, 0:8], c2t_sb[:])
            t2 = cp.tile([128, 8], F32)
            nc.vector.tensor_mul(t2[:], gates[:, 8:16], cnew[:])
            ct0 = cp.tile([128, 8], F32)
            nc.vector.tensor_add(ct0[:], t1[:], t2[:])

            # ---- LSTM tail --------------------------------------------
            mt_sb = cp.tile([128, 8], F32)
            nc.scalar.activation(mt_sb[:], stage2o[:, 40:48], AF.Tanh,
                                 scale=invS[:, 0:1])
            t3 = cp.tile([128, 8], F32)
            nc.vector.tensor_mul(t3[:], gates[:, 24:32], mt_sb[:])
            ct = cp.tile([128, 8], F32)
            nc.vector.tensor_add(ct[:], ct0[:], t3[:])
            tct = cp.tile([128, 8], F32)
            nc.scalar.activation(tct[:], ct[:], AF.Tanh)
            ht = cp.tile([128, 8], F32)
            nc.vector.tensor_mul(ht[:], gates[:, 16:24], tct[:])
            ht_r = cp.tile([128, 8], BF16)
            nc.vector.tensor_copy(ht_r[:], ht[:])

            # ---- A2C head: hh = relu(W_ih @ h_t + b_ih) ---------------
            # moving-operand form: p0/p1 banks reused, 16 N=512 matmuls
            for c in range(8):
                nc.tensor.matmul(p0[:], ht_r[:, c:c + 1],
                                 wiht_sb[:, c, 0:512],
                                 start=(c == 0), stop=(c == 7))
                nc.tensor.matmul(p1[:], ht_r[:, c:c + 1],
                                 wiht_sb[:, c, 512:1024],
                                 start=(c == 0), stop=(c == 7))
            hh_row = cp.tile([1, H], F32)
            nc.vector.tensor_copy(hh_row[0:1, 0:512], p0[:])
            nc.vector.tensor_copy(hh_row[0:1, 512:1024], p1[:])
            for n in range(8):
                nc.tensor.transpose(psum_mt[:, n:n + 1],
                                    hh_row[0:1, n * 128:(n + 1) * 128],
                                    ones_sb[0:1, 0:1])
            hhb_sb = cp.tile([128, 8], F32)
            nc.vector.tensor_add(hhb_sb[:], psum_mt[:], biht_sb[:])
            hh_sb = cp.tile([128, 8], F32)
            nc.scalar.activation(hh_sb[:], hhb_sb[:], AF.Relu)

            psum_av = pp.tile([1, 3], F32, tag="pre0")
            for c in range(8):
                nc.tensor.matmul(psum_av[:], hh_sb[:, c:c + 1],
                                 wact_sb[:, c * 3:(c + 1) * 3],
                                 start=(c == 0), stop=(c == 7))
            av = cp.tile([1, 3], F32)
            nc.vector.tensor_add(av[:], psum_av[:], bac_sb[:])

            # ---- outputs ----------------------------------------------
            out_sb = cp.tile([128, 16], F32)
            nc.vector.tensor_copy(out_sb[:, 0:8], ht[:])
            nc.vector.tensor_copy(out_sb[:, 8:16], ct[:])
            nc.sync.dma_start(out_hc[:], out_sb[:])
            nc.sync.dma_start(out_av[:], av[:])

    nc.compile()
    return nc


def _get_nc():
    if "nc" not in _CACHE:
        _CACHE["nc"] = _build()
    return _CACHE["nc"]


def _prep_in_maps(x_t, h, c, keys, vals, W_i2h, b_i2h, W_h2h, b_h2h,
                  W_ih, b_ih, W_actor, b_actor, W_critic, b_critic, pick_arm):
    f = np.float32
    x_t = np.asarray(x_t, f)
    h = np.asarray(h, f).reshape(-1)          # [H]
    c = np.asarray(c, f).reshape(-1)          # [H]
    keys = np.asarray(keys, f)
    vals = np.asarray(vals, f)

    pa = int(np.asarray(pick_arm))
    start = min(max(pa * RD, 0), IN_DIM - RD)  # jax dynamic_slice clamping
    q = x_t[0, start:start + RD]

    q_rep = np.ascontiguousarray(
        np.broadcast_to(np.tile(q, G), (128, G * RD)))

    b5 = (np.asarray(b_i2h, f) + np.asarray(b_h2h, f))
    b5t = np.ascontiguousarray(b5.reshape(40, 128).T)
    biht = np.ascontiguousarray(np.asarray(b_ih, f).reshape(8, 128).T)
    c2t = np.ascontiguousarray(c.reshape(8, 128).T)

    BF = ml_dtypes.bfloat16
    wiht = np.ascontiguousarray(
        np.asarray(W_ih, f).T.reshape(8, 128, H).transpose(1, 0, 2)
        .reshape(128, 8 * H)).astype(BF)
    wac = np.vstack([np.asarray(W_actor, f), np.asarray(W_critic, f)])  # [3,H]
    wact = np.ascontiguousarray(
        wac.T.reshape(8, 128, 3).transpose(1, 0, 2).reshape(128, 24))
    bac = np.concatenate([np.asarray(b_actor, f),
                          np.asarray(b_critic, f)]).reshape(1, 3)

    W_i2hT = np.ascontiguousarray(np.asarray(W_i2h, f).T).astype(np.float16)
    wxt_zero = np.zeros_like(W_i2hT)
    x_col = np.ascontiguousarray(x_t[0].reshape(IN_DIM, 1)).astype(np.float16)
    x_zero = np.zeros_like(x_col)

    in_maps = []
    for k in range(N_CORES):
        r0 = k * PER
        r1 = min(r0 + PER, D)
        n_valid = r1 - r0

        vals_p = np.zeros((PER, H), f)
        vals_p[:n_valid] = vals[r0:r1]
        vals_s = np.ascontiguousarray(
            vals_p.reshape(G, 128, H).transpose(1, 0, 2)
            .reshape(128, G * H)).astype(BF)
        keys_p = np.zeros((PER, RD), f)
        keys_p[:n_valid] = keys[r0:r1]
        keys_t = np.ascontiguousarray(
            keys_p.reshape(G, 128, RD).transpose(1, 0, 2).reshape(128, G * RD))
        idx = np.arange(G)[None, :] * 128 + np.arange(128)[:, None]
        mask = (idx < n_valid).astype(f)

        wht = np.ascontiguousarray(
            np.asarray(W_h2h, f)[:, k * 128:(k + 1) * 128].T).astype(np.float16)
        h_col = np.ascontiguousarray(
            h[k * 128:(k + 1) * 128].reshape(128, 1)).astype(np.float16)

        in_maps.append({
            "vals_s": vals_s,
            "keys_t": keys_t,
            "q_rep": q_rep,
            "mask": mask,
            "wht": wht,
            "wxt": W_i2hT if k == 0 else wxt_zero,
            "x_col": x_col if k == 0 else x_zero,
            "h_col": h_col,
            "c2t": c2t,
            "b5t": b5t,
            "biht": biht,
            "wiht": wiht,
            "wact": wact,
            "bac": bac,
        })
    return in_maps


def _postprocess(out_hc, out_av):
    h_t = np.ascontiguousarray(out_hc[:, 0:8].T).reshape(-1)
    c_t = np.ascontiguousarray(out_hc[:, 8:16].T).reshape(-1)
    logits = out_av[0, 0:2].astype(np.float32)
    v = np.float32(out_av[0, 2])
    m = logits.max()
    ex = np.exp(logits - m)
    pi = (ex / ex.sum()).astype(np.float32)
    a = int(np.argmax(np.log(pi) + GUMBEL))
    logp = np.float32(np.log(pi[a]))
    return np.concatenate([pi, [v], [logp], h_t, c_t]).astype(np.float32)


def kernel(**inputs) -> np.ndarray:
    nc = _get_nc()
    in_maps = _prep_in_maps(**inputs)
    res = run_bass_kernel_spmd(
        nc, in_maps, core_ids=list(range(N_CORES)),
        **_CACHE.get("run_kwargs", {}))
    _CACHE["last_results"] = res
    r0 = res.results[0]
    return _postprocess(r0["out_hc"], r0["out_av"])



# revision 14
# speedup vs baseline: 2.3271x; 2.3271x over previous
"""Trainium2 Bass kernel for CompositionalTwoArmedAgent (DND-LSTM A2C step).

Strategy (8 NeuronCores, SPMD, zero collectives):
  - The DND softmax weights w = softmax(cos(keys, q)) depend only on the
    tiny keys table (100000 x 10) and x_t, so the host computes them
    exactly (f32) and uploads the scaled weights in fp8 to every core.
  - vals (100000 x 1024, 400 MB f32) dominates HBM traffic. It is
    sharded COLUMN-wise: core k owns vals[:, 128k:128k+128] in fp8
    (12.85 MB/core) and computes its own 128-dim slice of
    m_t/c_t/h_t with no cross-core reduction at all.
  - The matvec p = w @ vals_slice streams vals through the PE array with
    fp8 DoubleRow matmuls: moving [128, 2, 512] consumes 8 row-chunks
    per instruction (2 k-tiles x 4 block-diagonal chunks packed into the
    512 free columns), 2x the bf16 column rate.  Two PSUM banks
    alternate; the 4 diagonal blocks are extracted at the end with
    [4,128]->[128,4] PE transposes.
  - The LSTM preact is computed per-core only for the 5 gate rows that
    core's slice needs (W rows {s, H+s, 2H+s, 3H+s, 4H+s}), so the
    1.25 MB/core weight load and the gate math also need no collective.
  - A2C head: each core outputs W_ih[:, slice] @ h_t_slice; the host
    sums the 8 partials, applies relu and the tiny (3 x 1024) actor/
    critic matvecs, the 2-class softmax and the fixed-key categorical
    sample (host postprocessing as in the original baseline).
"""

import ml_dtypes
import numpy as np

import concourse.bacc as bacc
import concourse.bass as bass
import concourse.mybir as mybir
import concourse.tile as tile
from concourse.bass_utils import run_bass_kernel_spmd

N_CORES = 8
D, RD, H, IN_DIM, A = 100000, 10, 1024, 14, 2
CH = 784               # 128-row chunks after padding (multiple of 8)
NT = CH // 8           # 98 DoubleRow matmuls (8 chunks each)
PAD_ROWS = CH * 128    # 100352
BLOCKS = [14] * 6 + [7, 4, 2, 1]   # groups (8 chunks) per vals DMA block
F32 = mybir.dt.float32
BF16 = mybir.dt.bfloat16
F16 = mybir.dt.float16
F8 = mybir.dt.float8e4
FP8 = ml_dtypes.float8_e4m3

# jax.random.gumbel(jax.random.key(1), (2,), float32) — fixed constants of the
# reference's categorical sample (verified against jax.random.categorical).
GUMBEL = np.array([0.5325072, -0.01641824], np.float32)

_CACHE = {}


def _input_specs():
    return [
        ("vals_s", [128, CH * 128], F8),   # chunk-tiled fp8 vals column-slice
        ("w_t", [128, NT * 64], F8),       # scaled softmax weights, [t, 2, 32]
                                           # layout (ISA needs >=32 stat cols)
        ("cst16", [128, 8 * 640 + 8], F16),  # [w5t(5120) | h_cols(8)]
        ("cstx", [IN_DIM, 641], F16),        # [wxt(640) | x_col(1)]
        ("cst32", [128, 11], F32),           # [b5t(5)|c2(1)|winv(1)|id4(4)]
        ("wiht", [128, H], BF16),            # W_ih[:, slice].T moving layout
    ]


def _build():
    nc = bacc.Bacc("TRN2", target_bir_lowering=False, debug=False,
                   num_devices=N_CORES)
    d = {name: nc.dram_tensor(name, shp, dt, kind="ExternalInput")
         for name, shp, dt in _input_specs()}
    out_hc = nc.dram_tensor("out_hc", [128, 2], F32, kind="ExternalOutput")
    out_hh = nc.dram_tensor("out_hh", [1, H], F32, kind="ExternalOutput")

    AF = mybir.ActivationFunctionType
    OP = mybir.AluOpType
    DR = mybir.MatmulPerfMode.DoubleRow

    with tile.TileContext(nc) as tc:
        with (
            tc.tile_pool(name="const", bufs=1) as cp,
            tc.tile_pool(name="vals", bufs=3) as vp,
            tc.tile_pool(name="ps", bufs=1, space="PSUM") as pp,
        ):
            # ---- persistent loads (scalar queue; w_t first) ------------
            w_sb = cp.tile([128, NT, 2, 32], F8)
            nc.scalar.dma_start(
                w_sb[:], d["w_t"][:].rearrange("p (t i m) -> p t i m",
                                               i=2, m=32))
            cst16 = cp.tile([128, 8 * 640 + 8], F16)
            nc.scalar.dma_start(cst16[:], d["cst16"][:])
            cstx = cp.tile([IN_DIM, 641], F16)
            nc.scalar.dma_start(cstx[:], d["cstx"][:])
            cst32 = cp.tile([128, 11], F32)
            nc.scalar.dma_start(cst32[:], d["cst32"][:])
            wiht_sb = cp.tile([128, H], BF16)
            nc.scalar.dma_start(wiht_sb[:], d["wiht"][:])

            ones11 = cp.tile([1, 1], F32)
            nc.vector.memset(ones11[:], 1.0)
            id4 = cst32[0:4, 7:11]   # 4x4 identity, uploaded with cst32

            # ---- preact partial: [x;h] @ W rows of this core's gates ---
            # moving-operand f16 matmuls: contraction over all 1024 h-dims
            # (8 chunk-columns of cst16 tail) + the 14-dim x part.
            preA = pp.tile([1, 512], F32, tag="preA")
            preB = pp.tile([1, 128], F32, tag="preB")
            for c in range(8):
                h_col = cst16[:, 8 * 640 + c:8 * 640 + c + 1]
                nc.tensor.matmul(preA[:], h_col, cst16[:, c * 640:c * 640 + 512],
                                 start=(c == 0), stop=False)
                nc.tensor.matmul(preB[:], h_col,
                                 cst16[:, c * 640 + 512:(c + 1) * 640],
                                 start=(c == 0), stop=False)
            nc.tensor.matmul(preA[:], cstx[:, 640:641], cstx[:, 0:512],
                             start=False, stop=True)
            nc.tensor.matmul(preB[:], cstx[:, 640:641], cstx[:, 512:640],
                             start=False, stop=True)

            # ---- big matvec: p = w @ vals_slice (fp8 DoubleRow) --------
            P0 = pp.tile([32, 512], F32, tag="mv0")
            P1 = pp.tile([32, 512], F32, tag="mv1")
            t = 0
            emitted_mid = False
            for nb in BLOCKS:
                v = vp.tile([128, nb, 2, 512], F8, tag="v")
                src = d["vals_s"][:, t * 1024:(t + nb) * 1024]
                nc.sync.dma_start(
                    v[:], src.rearrange("p (g i n) -> p g i n", i=2, n=512))
                for j in range(nb):
                    ps = P0 if (t % 2 == 0) else P1
                    nc.tensor.matmul(ps[:], w_sb[:, t], v[:, j],
                                     start=(t < 2), stop=(t >= NT - 2),
                                     perf_mode=DR)
                    t += 1
                if not emitted_mid:
                    # hide the preact transpose + gate math under the stream
                    emitted_mid = True
                    pre_row = cp.tile([1, 640], F32)
                    nc.vector.tensor_copy(pre_row[0:1, 0:512], preA[:])
                    nc.vector.tensor_copy(pre_row[0:1, 512:640], preB[:])
                    psum_preT = pp.tile([128, 5], F32, tag="preT")
                    for n in range(5):
                        nc.tensor.transpose(psum_preT[:, n:n + 1],
                                            pre_row[0:1, n * 128:(n + 1) * 128],
                                            ones11[:])
                    prefull = cp.tile([128, 5], F32)
                    nc.vector.tensor_add(prefull[:], psum_preT[:],
                                         cst32[:, 0:5])
                    th = cp.tile([128, 4], F32)
                    nc.scalar.activation(th[:], prefull[:, 0:4], AF.Tanh,
                                         scale=0.5)
                    gates = cp.tile([128, 4], F32)
                    nc.vector.tensor_scalar(gates[:], th[:], 0.5, 0.5,
                                            OP.mult, OP.add)
                    cnew = cp.tile([128, 1], F32)
                    nc.scalar.activation(cnew[:], prefull[:, 4:5], AF.Tanh)
                    t1 = cp.tile([128, 1], F32)
                    nc.vector.tensor_mul(t1[:], gates[:, 0:1], cst32[:, 5:6])
                    t2 = cp.tile([128, 1], F32)
                    nc.vector.tensor_mul(t2[:], gates[:, 1:2], cnew[:])
                    ct0 = cp.tile([128, 1], F32)
                    nc.vector.tensor_add(ct0[:], t1[:], t2[:])

            # ---- extract the 4 diagonal blocks of P0+P1 ----------------
            p1_sb = cp.tile([4, 512], F32)
            nc.vector.tensor_copy(p1_sb[:], P1[0:4, :])
            padd = cp.tile([4, 512], F32)
            nc.vector.tensor_add(padd[:], P0[0:4, :], p1_sb[:])
            psum_md = pp.tile([128, 16], F32, tag="md")
            for n in range(4):
                nc.tensor.transpose(psum_md[:, 4 * n:4 * n + 4],
                                    padd[0:4, n * 128:(n + 1) * 128],
                                    id4)
            # m_raw[h] = sum_m padd[m, 128m+h] = sum_n T_n[:, n] at col 5n
            md_sb = cp.tile([128, 16], F32)
            nc.vector.tensor_copy(md_sb[:], psum_md[:])
            s1 = cp.tile([128, 1], F32)
            nc.vector.tensor_add(s1[:], md_sb[:, 0:1], md_sb[:, 5:6])
            s2 = cp.tile([128, 1], F32)
            nc.vector.tensor_add(s2[:], md_sb[:, 10:11], md_sb[:, 15:16])
            m_raw = cp.tile([128, 1], F32)
            nc.vector.tensor_add(m_raw[:], s1[:], s2[:])

            # ---- LSTM tail (all [128, 1] column layout) ----------------
            mt = cp.tile([128, 1], F32)
            nc.scalar.activation(mt[:], m_raw[:], AF.Tanh,
                                 scale=cst32[:, 6:7])
            t3 = cp.tile([128, 1], F32)
            nc.vector.tensor_mul(t3[:], gates[:, 3:4], mt[:])
            ct = cp.tile([128, 1], F32)
            nc.vector.tensor_add(ct[:], ct0[:], t3[:])
            tct = cp.tile([128, 1], F32)
            nc.scalar.activation(tct[:], ct[:], AF.Tanh)
            ht = cp.tile([128, 1], F32)
            nc.vector.tensor_mul(ht[:], gates[:, 2:3], tct[:])

            # ---- A2C partial: hh_k = W_ih[:, slice] @ h_t_slice --------
            ht_bf = cp.tile([128, 1], BF16)
            nc.vector.tensor_copy(ht_bf[:], ht[:])
            hh0 = pp.tile([1, 512], F32, tag="mv0")
            hh1 = pp.tile([1, 512], F32, tag="mv1")
            nc.tensor.matmul(hh0[:], ht_bf[:], wiht_sb[:, 0:512])
            nc.tensor.matmul(hh1[:], ht_bf[:], wiht_sb[:, 512:1024])

            # ---- outputs -----------------------------------------------
            out_sb = cp.tile([128, 2], F32)
            nc.vector.tensor_copy(out_sb[:, 0:1], ht[:])
            nc.vector.tensor_copy(out_sb[:, 1:2], ct[:])
            hh_row = cp.tile([1, H], F32)
            nc.vector.tensor_copy(hh_row[0:1, 0:512], hh0[:])
            nc.vector.tensor_copy(hh_row[0:1, 512:1024], hh1[:])
            nc.sync.dma_start(out_hc[:], out_sb[:])
            nc.sync.dma_start(out_hh[:], hh_row[:])

    nc.compile()
    return nc


def _get_nc():
    if "nc" not in _CACHE:
        _CACHE["nc"] = _build()
    return _CACHE["nc"]


def _prep_in_maps(x_t, h, c, keys, vals, W_i2h, b_i2h, W_h2h, b_h2h,
                  W_ih, b_ih, W_actor, b_actor, W_critic, b_critic, pick_arm):
    f = np.float32
    x_t = np.asarray(x_t, f)
    h1 = np.asarray(h, f).reshape(-1)          # [H]
    c1 = np.asarray(c, f).reshape(-1)          # [H]
    keys = np.asarray(keys, f)
    vals = np.asarray(vals, f)
    W_i2h = np.asarray(W_i2h, f)
    W_h2h = np.asarray(W_h2h, f)
    W_ih = np.asarray(W_ih, f)

    # ---- host: exact softmax weights over the DND keys ------------------
    pa = int(np.asarray(pick_arm))
    start = min(max(pa * RD, 0), IN_DIM - RD)  # jax dynamic_slice clamping
    q = x_t[0, start:start + RD]
    dots = keys @ q
    kn = np.sqrt((keys * keys).sum(axis=1))
    qn = np.sqrt((q * q).sum())
    denom = np.maximum(kn * qn, np.float32(1e-8))
    s = dots / denom
    e = np.exp(s - s.max())
    w = e / e.sum()                            # [D], f32

    p2 = np.floor(np.log2(128.0 / max(float(w.max()), 1e-30)))
    wscale = np.float32(2.0 ** p2)
    winv = np.float32(1.0 / wscale)
    w_pad = np.zeros(PAD_ROWS, f)
    w_pad[:D] = w * wscale
    w_big = np.zeros((NT, 2, 32, 128), f)      # [t, ktile, stat_col, p]
    w_big[:, :, :4, :] = w_pad.reshape(NT, 2, 4, 128)
    w_t = np.ascontiguousarray(
        w_big.transpose(3, 0, 1, 2).reshape(128, NT * 64)).astype(FP8)

    # ---- vals: pad rows, fp8, chunk-tile, per-core column slices --------
    vals_f8 = np.zeros((PAD_ROWS, H), FP8)
    vals_f8[:D] = vals.astype(FP8)
    # [PAD_ROWS, H] -> [core, 128, CH * 128]
    vt = (vals_f8.reshape(CH, 128, N_CORES, 128)
          .transpose(2, 1, 0, 3).reshape(N_CORES, 128, CH * 128))

    b5 = np.asarray(b_i2h, f) + np.asarray(b_h2h, f)   # [5H]
    x_colx = np.concatenate(
        [np.zeros((IN_DIM, 640), f), x_t[0].reshape(IN_DIM, 1)], axis=1)

    in_maps = []
    for k in range(N_CORES):
        sl = slice(128 * k, 128 * (k + 1))
        rows = (np.arange(5)[:, None] * H + np.arange(128 * k, 128 * (k + 1))
                ).ravel()                                # [640] gate rows
        W5 = W_h2h[rows]                                 # [640, 1024]
        w5t = (W5.T.reshape(8, 128, 640).transpose(1, 0, 2)
               .reshape(128, 8 * 640))
        h_cols = h1.reshape(8, 128).T                    # [128, 8]
        cst16 = np.concatenate([w5t, h_cols], axis=1).astype(np.float16)

        cstx = x_colx.copy()
        cstx[:, 0:640] = W_i2h[rows].T                   # [14, 640]
        cstx = cstx.astype(np.float16)

        b5t = np.ascontiguousarray(b5[rows].reshape(5, 128).T)   # [128, 5]
        id4p = np.zeros((128, 4), f)
        id4p[:4, :4] = np.eye(4, dtype=f)
        cst32 = np.concatenate(
            [b5t, c1[sl].reshape(128, 1),
             np.full((128, 1), winv, f), id4p], axis=1).astype(f)

        wiht = np.ascontiguousarray(W_ih[:, sl].T).astype(ml_dtypes.bfloat16)

        in_maps.append({
            "vals_s": np.ascontiguousarray(vt[k]),
            "w_t": w_t,
            "cst16": np.ascontiguousarray(cst16),
            "cstx": np.ascontiguousarray(cstx),
            "cst32": np.ascontiguousarray(cst32),
            "wiht": wiht,
        })

    aux = {
        "b_ih": np.asarray(b_ih, f),
        "W_actor": np.asarray(W_actor, f),
        "b_actor": np.asarray(b_actor, f),
        "W_critic": np.asarray(W_critic, f),
        "b_critic": np.asarray(b_critic, f),
    }
    return in_maps, aux


def _postprocess(results, aux):
    h_t = np.concatenate([np.asarray(results[k]["out_hc"][:, 0], np.float32)
                          for k in range(N_CORES)])
    c_t = np.concatenate([np.asarray(results[k]["out_hc"][:, 1], np.float32)
                          for k in range(N_CORES)])
    hh = np.sum([np.asarray(results[k]["out_hh"][0], np.float32)
                 for k in range(N_CORES)], axis=0) + aux["b_ih"]
    hh = np.maximum(hh, 0.0)
    logits = aux["W_actor"] @ hh + aux["b_actor"]        # [A]
    v = aux["W_critic"] @ hh + aux["b_critic"]           # [1]
    m = logits.max()
    ex = np.exp(logits - m)
    pi = (ex / ex.sum()).astype(np.float32)
    a = int(np.argmax(np.log(pi) + GUMBEL))
    logp = np.float32(np.log(pi[a]))
    return np.concatenate([pi, v.astype(np.float32), [logp], h_t, c_t]
                          ).astype(np.float32)


def kernel(**inputs) -> np.ndarray:
    nc = _get_nc()
    in_maps, aux = _prep_in_maps(**inputs)
    res = run_bass_kernel_spmd(
        nc, in_maps, core_ids=list(range(N_CORES)),
        **_CACHE.get("run_kwargs", {}))
    _CACHE["last_results"] = res
    return _postprocess(res.results, aux)


# revision 36
# speedup vs baseline: 2.7296x; 1.1730x over previous
"""Trainium2 Bass kernel for CompositionalTwoArmedAgent (DND-LSTM A2C step).

Strategy (8 NeuronCores, SPMD, zero collectives):
  - The DND softmax weights w = softmax(cos(keys, q)) depend only on the
    tiny keys table (100000 x 10) and x_t, so the host computes them
    exactly (f32) and uploads the scaled weights in fp8 to every core.
  - vals (100000 x 1024, 400 MB f32) dominates HBM traffic. It is
    sharded COLUMN-wise: core k owns vals[:, 128k:128k+128] in fp8
    (12.85 MB/core) and computes its own 128-dim slice of
    m_t/c_t/h_t with no cross-core reduction at all.
  - The matvec p = w @ vals_slice streams vals through the PE array with
    fp8 DoubleRow matmuls: moving [128, 2, 512] consumes 8 row-chunks
    per instruction (2 k-tiles x 4 block-diagonal chunks packed into the
    512 free columns), 2x the bf16 column rate.  Two PSUM banks
    alternate; the 4 diagonal blocks are extracted at the end with
    [4,128]->[128,4] PE transposes.
  - The LSTM preact is computed per-core only for the 5 gate rows that
    core's slice needs (W rows {s, H+s, 2H+s, 3H+s, 4H+s}), so the
    1.25 MB/core weight load and the gate math also need no collective.
  - A2C head: each core outputs W_ih[:, slice] @ h_t_slice; the host
    sums the 8 partials, applies relu and the tiny (3 x 1024) actor/
    critic matvecs, the 2-class softmax and the fixed-key categorical
    sample (host postprocessing as in the original baseline).
"""

import ml_dtypes
import numpy as np

import concourse.bacc as bacc
import concourse.bass as bass
import concourse.mybir as mybir
import concourse.tile as tile
from concourse.bass_utils import run_bass_kernel_spmd

N_CORES = 8
D, RD, H, IN_DIM, A = 100000, 10, 1024, 14, 2
CH = 784               # 128-row chunks after padding (multiple of 8)
NT = CH // 8           # 98 DoubleRow matmuls (8 chunks each)
PAD_ROWS = CH * 128    # 100352
# groups (8 chunks) per vals DMA block: small first blocks so the first
# matmul fires early; small last blocks to shorten the drain.
BLOCKS = [2, 4, 8, 14, 14, 14, 14, 14, 6, 4, 2, 2]
W5_SCALE = 16.0        # fp8 range scaling for the preact weights
H_SCALE = 8.0          # fp8 range scaling for the h/x stationary vector
F32 = mybir.dt.float32
F32R = mybir.dt.float32r
BF16 = mybir.dt.bfloat16
F16 = mybir.dt.float16
F8 = mybir.dt.float8e4
FP8 = ml_dtypes.float8_e4m3

# jax.random.gumbel(jax.random.key(1), (2,), float32) — fixed constants of the
# reference's categorical sample (verified against jax.random.categorical).
GUMBEL = np.array([0.5325072, -0.01641824], np.float32)

_CACHE = {}


def _input_specs():
    return [
        ("vals_s", [128, CH * 128], F8),   # chunk-tiled fp8 vals column-slice
        ("w_t", [128, NT * 64], F8),       # scaled softmax weights, [t, 2, 32]
                                           # layout (ISA needs >=32 stat cols)
        ("cst16", [128, 8 * 640 + 8], F16),  # [w5t(5120) | h_cols(8)]
        ("cstx", [IN_DIM, 641], F16),        # [wxt(640) | x_col(1)]
        ("cst32", [128, 23], F32),         # [b5t(5)|c2(1)|winv(1)|perm4x4(16)]
        ("wiht", [128, H], F32R),          # W_ih[:, slice].T moving layout
    ]


def _build():
    nc = bacc.Bacc("TRN2", target_bir_lowering=False, debug=False,
                   num_devices=N_CORES)
    d = {name: nc.dram_tensor(name, shp, dt, kind="ExternalInput")
         for name, shp, dt in _input_specs()}
    out_hc = nc.dram_tensor("out_hc", [128, 2], F32, kind="ExternalOutput")
    out_hh = nc.dram_tensor("out_hh", [1, H], F32, kind="ExternalOutput")

    AF = mybir.ActivationFunctionType
    OP = mybir.AluOpType
    DR = mybir.MatmulPerfMode.DoubleRow

    with tile.TileContext(nc) as tc:
        with (
            tc.tile_pool(name="const", bufs=1) as cp,
            tc.tile_pool(name="vals", bufs=3) as vp,
            tc.tile_pool(name="ps", bufs=1, space="PSUM") as pp,
        ):
            # ---- persistent loads (scalar queue; w_t first) ------------
            w_sb = cp.tile([128, NT, 2, 32], F8)
            nc.scalar.dma_start(
                w_sb[:], d["w_t"][:].rearrange("p (t i m) -> p t i m",
                                               i=2, m=32))
            cst32 = cp.tile([128, 23], F32)
            nc.scalar.dma_start(cst32[:], d["cst32"][:])
            cst16 = cp.tile([128, 8 * 640 + 8], F16)
            nc.scalar.dma_start(cst16[:], d["cst16"][:])
            cstx = cp.tile([IN_DIM, 641], F16)
            nc.scalar.dma_start(cstx[:], d["cstx"][:])
            wiht_sb = cp.tile([128, H], F32R)
            nc.scalar.dma_start(wiht_sb[:], d["wiht"][:])

            ones11 = cp.tile([1, 1], F32)
            nc.vector.memset(ones11[:], 1.0)

            # ---- big matvec: p = w @ vals_slice (fp8 DoubleRow) --------
            P0 = pp.tile([32, 512], F32, tag="mv0")
            P1 = pp.tile([32, 512], F32, tag="mv1")
            t = 0
            for bi, nb in enumerate(BLOCKS):
                v = vp.tile([128, nb, 2, 512], F8, tag="v")
                src = d["vals_s"][:, t * 1024:(t + nb) * 1024]
                nc.sync.dma_start(
                    v[:], src.rearrange("p (g i n) -> p g i n", i=2, n=512))
                for j in range(nb):
                    ps = P0 if (t % 2 == 0) else P1
                    nc.tensor.matmul(ps[:], w_sb[:, t], v[:, j],
                                     start=(t < 2), stop=(t >= NT - 2),
                                     perf_mode=DR)
                    t += 1
                if bi == 2:
                    # preact (f16 moving-operand matmuls) + gate math,
                    # hidden inside the stream while DMA is ahead
                    preA = pp.tile([1, 512], F32, tag="preA")
                    preB = pp.tile([1, 128], F32, tag="preB")
                    for c in range(8):
                        h_col = cst16[:, 8 * 640 + c:8 * 640 + c + 1]
                        nc.tensor.matmul(preA[:], h_col,
                                         cst16[:, c * 640:c * 640 + 512],
                                         start=(c == 0), stop=False)
                        nc.tensor.matmul(preB[:], h_col,
                                         cst16[:, c * 640 + 512:(c + 1) * 640],
                                         start=(c == 0), stop=False)
                    nc.tensor.matmul(preA[:], cstx[:, 640:641],
                                     cstx[:, 0:512], start=False, stop=True)
                    nc.tensor.matmul(preB[:], cstx[:, 640:641],
                                     cstx[:, 512:640], start=False, stop=True)
                    pre_row = cp.tile([1, 640], F32)
                    nc.vector.tensor_copy(pre_row[0:1, 0:512], preA[:])
                    nc.vector.tensor_copy(pre_row[0:1, 512:640], preB[:])
                    psum_preT = pp.tile([128, 5], F32, tag="preT")
                    for n in range(5):
                        nc.tensor.transpose(psum_preT[:, n:n + 1],
                                            pre_row[0:1, n * 128:(n + 1) * 128],
                                            ones11[:])
                    prefull = cp.tile([128, 5], F32)
                    nc.vector.tensor_add(prefull[:], psum_preT[:],
                                         cst32[:, 0:5])
                    th = cp.tile([128, 4], F32)
                    nc.scalar.activation(th[:], prefull[:, 0:4], AF.Tanh,
                                         scale=0.5)
                    gates = cp.tile([128, 4], F32)
                    nc.vector.tensor_scalar(gates[:], th[:], 0.5, 0.5,
                                            OP.mult, OP.add)
                    cnew = cp.tile([128, 1], F32)
                    nc.scalar.activation(cnew[:], prefull[:, 4:5], AF.Tanh)
                    t1 = cp.tile([128, 1], F32)
                    nc.vector.tensor_mul(t1[:], gates[:, 0:1], cst32[:, 5:6])
                    t2 = cp.tile([128, 1], F32)
                    nc.vector.tensor_mul(t2[:], gates[:, 1:2], cnew[:])
                    ct0 = cp.tile([128, 1], F32)
                    nc.vector.tensor_add(ct0[:], t1[:], t2[:])

            # ---- extract the 4 diagonal blocks of P0+P1 ----------------
            p1_sb = cp.tile([4, 512], F32)
            nc.vector.tensor_copy(p1_sb[:], P1[0:4, :])
            padd = cp.tile([4, 512], F32)
            nc.vector.tensor_add(padd[:], P0[0:4, :], p1_sb[:])
            # 4 accumulating [4,128]->[128,4] transposes, each through a
            # cyclic permutation, land the diagonal sum in psum column 0
            md4 = pp.tile([128, 4], F32, tag="md")
            for n in range(4):
                nc.tensor.matmul(md4[:], padd[0:4, n * 128:(n + 1) * 128],
                                 cst32[0:4, 7 + 4 * n:11 + 4 * n],
                                 is_transpose=True,
                                 start=(n == 0), stop=(n == 3))

            # ---- LSTM tail (all [128, 1] column layout) ----------------
            mt = cp.tile([128, 1], F32)
            nc.scalar.activation(mt[:], md4[:, 0:1], AF.Tanh,
                                 scale=cst32[:, 6:7])
            t3 = cp.tile([128, 1], F32)
            nc.vector.tensor_mul(t3[:], gates[:, 3:4], mt[:])
            ct = cp.tile([128, 1], F32)
            nc.vector.tensor_add(ct[:], ct0[:], t3[:])
            tct = cp.tile([128, 1], F32)
            nc.scalar.activation(tct[:], ct[:], AF.Tanh)
            ht = cp.tile([128, 2], F32)
            nc.vector.tensor_mul(ht[:, 0:1], gates[:, 2:3], tct[:])
            nc.vector.tensor_copy(ht[:, 1:2], ct[:])

            # ---- A2C partial: hh_k = W_ih[:, slice] @ h_t_slice --------
            ht_r = cp.tile([128, 1], F32R)
            nc.vector.tensor_copy(ht_r[:], ht[:, 0:1])
            hh0 = pp.tile([1, 512], F32, tag="preA")
            hh1 = pp.tile([1, 512], F32, tag="preB")
            nc.tensor.matmul(hh0[:], ht_r[:], wiht_sb[:, 0:512])
            nc.tensor.matmul(hh1[:], ht_r[:], wiht_sb[:, 512:1024])

            # ---- outputs (out_hc = [h_t | c_t]) ------------------------
            nc.sync.dma_start(out_hc[:], ht[:])
            hh_row = cp.tile([1, H], F32)
            nc.vector.tensor_copy(hh_row[0:1, 0:512], hh0[:])
            nc.scalar.activation(hh_row[0:1, 512:1024], hh1[:], AF.Copy)
            nc.sync.dma_start(out_hh[:], hh_row[:])

    nc.compile()
    return nc


def _get_nc():
    if "nc" not in _CACHE:
        _CACHE["nc"] = _build()
    return _CACHE["nc"]


def _prep_in_maps(x_t, h, c, keys, vals, W_i2h, b_i2h, W_h2h, b_h2h,
                  W_ih, b_ih, W_actor, b_actor, W_critic, b_critic, pick_arm):
    f = np.float32
    x_t = np.asarray(x_t, f)
    h1 = np.asarray(h, f).reshape(-1)          # [H]
    c1 = np.asarray(c, f).reshape(-1)          # [H]
    keys = np.asarray(keys, f)
    vals = np.asarray(vals, f)
    W_i2h = np.asarray(W_i2h, f)
    W_h2h = np.asarray(W_h2h, f)
    W_ih = np.asarray(W_ih, f)

    # ---- host: exact softmax weights over the DND keys ------------------
    pa = int(np.asarray(pick_arm))
    start = min(max(pa * RD, 0), IN_DIM - RD)  # jax dynamic_slice clamping
    q = x_t[0, start:start + RD]
    dots = keys @ q
    kn = np.sqrt((keys * keys).sum(axis=1))
    qn = np.sqrt((q * q).sum())
    denom = np.maximum(kn * qn, np.float32(1e-8))
    s = dots / denom
    e = np.exp(s - s.max())
    w = e / e.sum()                            # [D], f32

    p2 = np.floor(np.log2(128.0 / max(float(w.max()), 1e-30)))
    wscale = np.float32(2.0 ** p2)
    winv = np.float32(1.0 / wscale)
    w_pad = np.zeros(PAD_ROWS, f)
    w_pad[:D] = w * wscale
    w_big = np.zeros((NT, 2, 32, 128), f)      # [t, ktile, stat_col, p]
    w_big[:, :, :4, :] = w_pad.reshape(NT, 2, 4, 128)
    w_t = np.ascontiguousarray(
        w_big.transpose(3, 0, 1, 2).reshape(128, NT * 64)).astype(FP8)

    # ---- vals: pad rows, fp8, chunk-tile, per-core column slices --------
    vals_f8 = np.zeros((PAD_ROWS, H), FP8)
    vals_f8[:D] = vals.astype(FP8)
    # [PAD_ROWS, H] -> [core, 128, CH * 128]
    vt = (vals_f8.reshape(CH, 128, N_CORES, 128)
          .transpose(2, 1, 0, 3).reshape(N_CORES, 128, CH * 128))

    b5 = np.asarray(b_i2h, f) + np.asarray(b_h2h, f)   # [5H]
    x_colx = np.concatenate(
        [np.zeros((IN_DIM, 640), f), x_t[0].reshape(IN_DIM, 1)], axis=1)
    h_cols = h1.reshape(8, 128).T                      # [128, 8]

    in_maps = []
    for k in range(N_CORES):
        sl = slice(128 * k, 128 * (k + 1))
        rows = (np.arange(5)[:, None] * H + np.arange(128 * k, 128 * (k + 1))
                ).ravel()                                # [640] gate rows
        w5t = (W_h2h[rows].T.reshape(8, 128, 640).transpose(1, 0, 2)
               .reshape(128, 8 * 640))
        cst16 = np.concatenate([w5t, h_cols], axis=1).astype(np.float16)
        cstx = x_colx.copy()
        cstx[:, 0:640] = W_i2h[rows].T                   # [14, 640]
        cstx = cstx.astype(np.float16)

        b5t = np.ascontiguousarray(b5[rows].reshape(5, 128).T)   # [128, 5]
        # perm n: P[m, j] = 1 iff m == (j + n) % 4  (diag sum -> column 0)
        perms = np.zeros((128, 16), f)
        for n in range(4):
            for j in range(4):
                perms[(j + n) % 4, 4 * n + j] = 1.0
        cst32 = np.concatenate(
            [b5t, c1[sl].reshape(128, 1),
             np.full((128, 1), winv, f), perms], axis=1).astype(f)

        wiht = np.ascontiguousarray(W_ih[:, sl].T).astype(f)

        in_maps.append({
            "vals_s": np.ascontiguousarray(vt[k]),
            "w_t": w_t,
            "cst16": np.ascontiguousarray(cst16),
            "cstx": np.ascontiguousarray(cstx),
            "cst32": np.ascontiguousarray(cst32),
            "wiht": wiht,
        })

    aux = {
        "b_ih": np.asarray(b_ih, f),
        "W_actor": np.asarray(W_actor, f),
        "b_actor": np.asarray(b_actor, f),
        "W_critic": np.asarray(W_critic, f),
        "b_critic": np.asarray(b_critic, f),
    }
    return in_maps, aux


def _postprocess(results, aux):
    h_t = np.concatenate([np.asarray(results[k]["out_hc"][:, 0], np.float32)
                          for k in range(N_CORES)])
    c_t = np.concatenate([np.asarray(results[k]["out_hc"][:, 1], np.float32)
                          for k in range(N_CORES)])
    hh = np.sum([np.asarray(results[k]["out_hh"][0], np.float32)
                 for k in range(N_CORES)], axis=0) + aux["b_ih"]
    hh = np.maximum(hh, 0.0)
    logits = aux["W_actor"] @ hh + aux["b_actor"]        # [A]
    v = aux["W_critic"] @ hh + aux["b_critic"]           # [1]
    m = logits.max()
    ex = np.exp(logits - m)
    pi = (ex / ex.sum()).astype(np.float32)
    a = int(np.argmax(np.log(pi) + GUMBEL))
    logp = np.float32(np.log(pi[a]))
    return np.concatenate([pi, v.astype(np.float32), [logp], h_t, c_t]
                          ).astype(np.float32)


def kernel(**inputs) -> np.ndarray:
    nc = _get_nc()
    in_maps, aux = _prep_in_maps(**inputs)
    res = run_bass_kernel_spmd(
        nc, in_maps, core_ids=list(range(N_CORES)),
        **_CACHE.get("run_kwargs", {}))
    _CACHE["last_results"] = res
    return _postprocess(res.results, aux)
